# revision 1
# baseline (speedup 1.0000x reference)
"""Trainium2 Bass kernel for nn_DSVDD (retrieval_knn).

Math (per batch b):
  phi = W @ p_b + bias            [DIM, HW]    (1x1 conv)
  sqdist[i,j] = ||phi_i||^2 + ||C_j||^2 - 2 phi_i . C_j
  top-3 smallest distances d0<=d1<=d2  ->  w0 = 1/(1+exp(d0-d1)+exp(d0-d2))
  score[i] = w0 * d0

Device strategy (8 cores, data-parallel over (batch, HW-half)):
  Y[i,j] = 2 phi_i . C_j - ||C_j||^2.  The 2C part runs as fp32r PE matmuls;
  the -c_j correction is materialized once per j-slice ([128, js] via a
  ones-matmul on the replicated -c/128 block) and applied by a DVE add.
  top-3 smallest sqdist == top-3 largest Y (f_i = ||phi_i||^2 common per row).
  DVE max8 finds the top-8 largest Y per row in one instruction; streamed
  merge over j-slices.  f_i via ones-matmuls over Squared phi (deferred one
  conv step so they never stall the PE).  Tail (sqrt, softmin) on ACT/DVE.
"""
import sys

sys.path.insert(0, "/opt/trn_rl_repo")

import numpy as np

B, DIM, H, W_ = 4, 1792, 56, 56
HW = H * W_            # 3136
P = 3136               # prototypes
NCORES = 8
HALF = HW // 2         # 1568 positions per core
KC = DIM // 128        # 14 contraction chunks
KCH = KC // 2          # 7 (p tiles split in halves for early start)
KCC = KC + 1           # 15 chunks in cb (incl. replicated -c/128 block)
IB = 392               # conv i-block (moving cols)
NIB = HALF // IB       # 4
JSLICES = [256, 480, 480, 480, 480, 480, 480]   # G-phase j-slices (sum 3136)
NIT = 13               # i-tiles: 12 full + 1 ragged(32)
LAST_W = HALF - 12 * 128   # 32

_cache = {}


def _build_program():
    import concourse.tile as tile
    from concourse import bacc, mybir

    F32 = mybir.dt.float32
    F32R = mybir.dt.float32r
    AF = mybir.ActivationFunctionType
    ALU = mybir.AluOpType
    AX = mybir.AxisListType

    nc = bacc.Bacc("TRN2", target_bir_lowering=False, debug=False)

    pT_d = nc.dram_tensor("pT", [DIM, HALF], F32R, kind="ExternalInput")
    wt_d = nc.dram_tensor("wt", [DIM, DIM], F32R, kind="ExternalInput")   # W^T
    cb_d = nc.dram_tensor("cb", [KCC * 128, P], F32R, kind="ExternalInput")
    bias_d = nc.dram_tensor("bias", [DIM], F32, kind="ExternalInput")
    onec_d = nc.dram_tensor("onec", [128, 1], F32R, kind="ExternalInput")
    oner_d = nc.dram_tensor("oner", [1, 128], F32R, kind="ExternalInput")
    ones2_d = nc.dram_tensor("ones2", [128, 128], F32R, kind="ExternalInput")
    score_d = nc.dram_tensor("score", [128, NIT], F32, kind="ExternalOutput")

    with tile.TileContext(nc) as tc:
        with (
            tc.tile_pool(name="persist", bufs=1) as persist,
            tc.tile_pool(name="cbp0", bufs=1) as cbp0,
        ):
            phi = persist.tile([128, KC, HALF], F32R)
            bias_col = persist.tile([128, KC], F32)
            onec = persist.tile([128, 1], F32R)
            oner = persist.tile([1, 128], F32R)
            ones2 = persist.tile([128, 128], F32R)
            f_row = persist.tile([1, HALF], F32)
            f_col = persist.tile([128, NIT], F32)
            runA = persist.tile([128, NIT, 8], F32)
            score_col = persist.tile([128, NIT], F32)

            # ------------- conv phase: phi = W @ p + b, f = ||phi||^2 -------
            with (
                tc.tile_pool(name="pp", bufs=6) as pp,
                tc.tile_pool(name="wtp", bufs=3) as wtp,
                tc.tile_pool(name="sqp", bufs=4) as sqp,
                tc.tile_pool(name="cps", bufs=4, space="PSUM") as cps,
                tc.tile_pool(name="fps", bufs=1, space="PSUM") as fps,
            ):
                f_ps = [fps.tile([1, IB], F32, name=f"fp{ib}", tag=f"f{ib}")
                        for ib in range(NIB)]

                def load_wt(dcg):
                    t = wtp.tile([128, KC, 128], F32R, name="wt_t")
                    nc.sync.dma_start(
                        t[:],
                        wt_d[:, dcg * 128:(dcg + 1) * 128].rearrange(
                            "(cc p) d -> p cc d", p=128),
                    )
                    return t

                def load_phalf(ib, h):
                    t = pp.tile([128, KCH, IB], F32R, name=f"pq{ib}{h}",
                                tag="pq")
                    nc.sync.dma_start(
                        t[:],
                        pT_d[h * KCH * 128:(h + 1) * KCH * 128,
                             ib * IB:(ib + 1) * IB].rearrange(
                            "(cc p) i -> p cc i", p=128),
                    )
                    return t

                # startup-critical loads first: wt chunk 0, then p halves
                dcg_seq = list(range(KC)) + list(reversed(range(KC)))  # snake
                wt_tiles = {0: load_wt(dcg_seq[0])}
                wt_issued = 1

                def wt_prefetch(upto):
                    nonlocal wt_issued
                    while wt_issued < min(upto, 2 * KC):
                        if dcg_seq[wt_issued] == dcg_seq[wt_issued - 1]:
                            # snake turn: same chunk again, reuse the tile
                            wt_tiles[wt_issued] = wt_tiles[wt_issued - 1]
                        else:
                            wt_tiles[wt_issued] = load_wt(dcg_seq[wt_issued])
                        wt_issued += 1

                # PE warmup: dummy matmuls keep HAM's activity monitor hot
                # while the first real DMAs land, so conv starts at 2.4 GHz.
                warm = pp.tile([128, 512], F32R, name="warm", tag="warm", bufs=1)
                nc.vector.memset(warm[:].bitcast(F32), 1.0)
                wps = cps.tile([128, 512], F32, name="wps", tag="acc")
                for _ in range(68):
                    nc.tensor.matmul(wps[:], warm[:, 0:128], warm[:],
                                     start=True, stop=True)

                cb0_t = None
                small_dmas_done = False
                pending_f = []
                for sub in range(2):
                    p_t = {}
                    for ib in (2 * sub, 2 * sub + 1):
                        p_t[ib] = [load_phalf(ib, 0), load_phalf(ib, 1)]
                    if not small_dmas_done:
                        small_dmas_done = True
                        nc.sync.dma_start(
                            bias_col[:],
                            bias_d.rearrange("(g p) -> p g", p=128))
                        nc.sync.dma_start(onec[:], onec_d[:])
                        nc.sync.dma_start(oner[:], oner_d[:])
                        nc.sync.dma_start(ones2[:], ones2_d[:])
                    for dcg_i in range(KC):
                        pos = sub * KC + dcg_i
                        dcg = dcg_seq[pos]
                        wt_t = wt_tiles.pop(pos)
                        wt_prefetch(pos + 3)
                        for k, ib in enumerate((2 * sub, 2 * sub + 1)):
                            if k == 1 and pending_f:
                                # deferred f matmuls: deps long satisfied
                                for args, kw in pending_f:
                                    nc.tensor.matmul(*args, **kw)
                                pending_f = []
                            acc = cps.tile([128, IB], F32)
                            for cc in range(KC):
                                nc.tensor.matmul(
                                    acc[:],
                                    wt_t[:, cc, :],
                                    p_t[ib][cc // KCH][:, cc % KCH, :],
                                    start=(cc == 0),
                                    stop=(cc == KC - 1),
                                )
                            isl = slice(ib * IB, (ib + 1) * IB)
                            # phi = psum + bias (rounded to fp32r)
                            nc.scalar.activation(
                                phi[:, dcg, isl], acc[:], AF.Identity,
                                bias=bias_col[:, dcg:dcg + 1],
                            )
                            # phi2 = (psum + bias)^2
                            sq = sqp.tile([128, IB], F32R)
                            nc.scalar.activation(
                                sq[:], acc[:], AF.Square,
                                bias=bias_col[:, dcg:dcg + 1],
                            )
                            pending_f.append((
                                (f_ps[ib][:], onec[:], sq[:]),
                                dict(start=(dcg_i == 0), stop=(dcg_i == KC - 1)),
                            ))
                    if sub == 0:
                        # prefetch first G slice mid-conv
                        j0 = JSLICES[0]
                        cb0_t = cbp0.tile([128, KCC, j0], F32R)
                        nc.sync.dma_start(
                            cb0_t[:],
                            cb_d[:, 0:j0].rearrange("(cc p) j -> p cc j",
                                                    p=128),
                        )
                for args, kw in pending_f:
                    nc.tensor.matmul(*args, **kw)
                pending_f = []
                for ib in range(NIB):
                    nc.vector.tensor_copy(
                        f_row[:, ib * IB:(ib + 1) * IB], f_ps[ib][:]
                    )

            # ------------- f relayout: [1, 1568] -> [128, 13] ---------------
            with tc.tile_pool(name="ftp", bufs=2, space="PSUM") as ftp:
                ft = ftp.tile([128, NIT], F32)
                for it in range(NIT):
                    w = 128 if it < 12 else LAST_W
                    nc.tensor.transpose(
                        ft[0:w, it:it + 1],
                        f_row[:, it * 128:it * 128 + w],
                        oner[0:1, 0:1].bitcast(F32),
                    )
                nc.scalar.activation(f_col[:], ft[:], AF.Copy)

            # ------------- G phase: Y = 2 phi.C - c, streamed top-8 ---------
            with (
                tc.tile_pool(name="cbp", bufs=2) as cbp,
                tc.tile_pool(name="cbcp", bufs=2) as cbcp,
                tc.tile_pool(name="ysb", bufs=4) as ysb,
                tc.tile_pool(name="mrg", bufs=4) as mrg,
                tc.tile_pool(name="yps", bufs=8, space="PSUM") as yps,
            ):
                joff = [0]
                for js in range(1, len(JSLICES)):
                    joff.append(joff[-1] + JSLICES[js - 1])

                for js in range(len(JSLICES)):
                    w_js = JSLICES[js]
                    jsl = slice(joff[js], joff[js] + w_js)
                    if js == 0:
                        cb_t = cb0_t
                    else:
                        cb_t = cbp.tile([128, KCC, w_js], F32R, name="cb_t",
                                        tag="cb")
                        nc.sync.dma_start(
                            cb_t[:],
                            cb_d[:, jsl].rearrange("(cc p) j -> p cc j",
                                                   p=128),
                        )
                    # materialize -c for this slice: ones2 @ (-c/128 block)
                    cps_t = yps.tile([128, 512], F32, name="y", tag="y")
                    nc.tensor.matmul(cps_t[:, 0:w_js], ones2[:],
                                     cb_t[:, KC, :], start=True, stop=True)
                    cbc_t = cbcp.tile([128, 512], F32, name="cbc_t")
                    nc.scalar.activation(cbc_t[:, 0:w_js], cps_t[:, 0:w_js],
                                         AF.Copy)
                    for it in range(NIT):
                        w = 128 if it < 12 else LAST_W
                        i0 = it * 128
                        y = yps.tile([128, 512], F32, name="y", tag="y")
                        for cc in range(KC):
                            nc.tensor.matmul(
                                y[0:w, 0:w_js],
                                phi[:, cc, i0:i0 + w],
                                cb_t[:, cc, :],
                                start=(cc == 0),
                                stop=(cc == KC - 1),
                            )
                        ys = ysb.tile([128, 512], F32, name="ys", tag="ys")
                        nc.vector.tensor_tensor(
                            ys[0:w, 0:w_js], y[0:w, 0:w_js],
                            cbc_t[0:w, 0:w_js], ALU.add,
                        )
                        if js == 0:
                            nc.vector.max(runA[0:w, it, :], ys[0:w, 0:w_js])
                        else:
                            m = mrg.tile([128, 16], F32)
                            nc.vector.tensor_copy(m[0:w, 0:8], runA[0:w, it, :])
                            nc.vector.max(m[0:w, 8:16], ys[0:w, 0:w_js])
                            nc.vector.max(runA[0:w, it, :], m[0:w, :])

                # ------------- tail: sqrt + softmin weight -------------------
                with tc.tile_pool(name="tails", bufs=4) as tails:
                    for it in range(NIT):
                        w = 128 if it < 12 else LAST_W
                        d3 = tails.tile([128, 3], F32, tag="d3")
                        nc.scalar.activation(
                            d3[0:w, :], runA[0:w, it, 0:3], AF.Sqrt,
                            bias=f_col[0:w, it:it + 1], scale=-1.0,
                        )
                        dd = tails.tile([128, 3], F32, tag="dd")
                        nc.vector.tensor_scalar(
                            dd[0:w, :], d3[0:w, :], d3[0:w, 0:1], None,
                            ALU.subtract,
                        )
                        ee = tails.tile([128, 3], F32, tag="ee")
                        nc.scalar.activation(ee[0:w, :], dd[0:w, :], AF.Exp,
                                             scale=-1.0)
                        ss = tails.tile([128, 1], F32, tag="ss")
                        nc.vector.tensor_reduce(ss[0:w, :], ee[0:w, :], AX.X,
                                                ALU.add)
                        rr = tails.tile([128, 1], F32, tag="rr")
                        nc.vector.reciprocal(rr[0:w, :], ss[0:w, :])
                        nc.vector.tensor_scalar(
                            score_col[0:w, it:it + 1], d3[0:w, 0:1],
                            rr[0:w, 0:1], None, ALU.mult,
                        )
            nc.sync.dma_start(score_d[:], score_col[:])

    nc.compile()
    return nc


def _get_program():
    if "nc" not in _cache:
        _cache["nc"] = _build_program()
    return _cache["nc"]


def kernel(p, W, b, C):
    from concourse.bass_utils import run_bass_kernel_spmd

    nc = _get_program()

    p = np.ascontiguousarray(np.asarray(p, dtype=np.float32))
    W = np.asarray(W, dtype=np.float32)
    b = np.ascontiguousarray(np.asarray(b, dtype=np.float32))
    C = np.ascontiguousarray(np.asarray(C, dtype=np.float32))

    wt = np.ascontiguousarray(W.T)                                # [c, d]
    cn = np.sum(C.astype(np.float64) * C, axis=0).astype(np.float32)
    cblock = np.broadcast_to((-cn / 128.0)[None, :], (128, P))
    cb = np.ascontiguousarray(
        np.concatenate([2.0 * C, cblock], axis=0)                 # [1920, P]
    )
    onec = np.ones((128, 1), dtype=np.float32)
    oner = np.ones((1, 128), dtype=np.float32)
    ones2 = np.ones((128, 128), dtype=np.float32)

    p_flat = p.reshape(B, DIM, HW)
    in_maps = []
    for core in range(NCORES):
        bidx, half = divmod(core, 2)
        pT = np.ascontiguousarray(p_flat[bidx, :, half * HALF:(half + 1) * HALF])
        in_maps.append({
            "pT": pT, "wt": wt, "cb": cb, "bias": b,
            "onec": onec, "oner": oner, "ones2": ones2,
        })

    _cache["last_in_maps"] = in_maps
    res = run_bass_kernel_spmd(nc, in_maps, list(range(NCORES)))
    _cache["last_result"] = res

    return assemble_output(per_core=[res.results[c]["score"] for c in range(NCORES)])


def assemble_output(per_core=None, res_concat=None):
    if per_core is None:
        sc_all = res_concat["score"]                              # [8*128, 13]
        per_core = [sc_all[c * 128:(c + 1) * 128] for c in range(NCORES)]
    out = np.empty((B, 1, H, W_), dtype=np.float32)
    for core in range(NCORES):
        bidx, half = divmod(core, 2)
        sc = per_core[core]                                       # [128, 13]
        flat = np.empty(HALF, dtype=np.float32)
        flat[:12 * 128] = sc[:, :12].T.reshape(-1)
        flat[12 * 128:] = sc[:LAST_W, 12]
        out.reshape(B, 1, HW)[bidx, 0, half * HALF:(half + 1) * HALF] = flat
    return out



# revision 3
# speedup vs baseline: 1.7983x; 1.7983x over previous
"""Trainium2 Bass kernel for nn_DSVDD (retrieval_knn), fp8 DoubleRow version.

Math (per batch b):
  phi = W @ p_b + bias            [DIM, HW]    (1x1 conv)
  sqdist[i,j] = ||phi_i||^2 + ||C_j||^2 - 2 phi_i . C_j
  top-3 smallest distances d0<=d1<=d2  ->  w0 = 1/(1+exp(d0-d1)+exp(d0-d2))
  score[i] = w0 * d0

Device strategy (8 cores, data-parallel over (batch, HW-half)):
  Both GEMMs run as fp8e4 DoubleRow matmuls (2 MAC/PE/cycle): host quantizes
  p*16, W*1024, 2C*512 to e4m3 (all well under the TRN 240 cap).  The
  -||c_j||^2 correction rides inside the distance matmul as an extra
  DoubleRow contraction pair: phi planes 14/15 are the constant 16.0 and cb
  planes 14/15 hold e4m3(-2*||c_j||^2) replicated, so psum = 8192*Y with
  Y = 2 phi.c - c and no per-tile vector add is needed.  top-3 smallest
  sqdist == top-3 largest Y (f_i common per row).  DVE max8 collects the
  top-8 of each 448..512-wide j-window into runAll; one final max8 per
  i-tile merges the windows.  f_i = ||phi||^2 comes from DVE squares of the
  quantized phi (consistent-f) reduced by ones-matmuls, deferred one conv
  step so they never stall the PE.  Tail (sqrt, softmin) on ACT/DVE.
"""
import sys

sys.path.insert(0, "/opt/trn_rl_repo")

import numpy as np

B, DIM, H, W_ = 4, 1792, 56, 56
HW = H * W_            # 3136
P = 3136               # prototypes
NCORES = 8
HALF = HW // 2         # 1568 positions per core
KC = DIM // 128        # 14 contraction chunks
KPAIR = KC // 2        # 7 DoubleRow pairs in the conv
KCB = KC + 2           # 16 planes in phi/cb (incl. the -c correction pair)
GPAIR = KCB // 2       # 8 DoubleRow pairs in the distance matmul
IB = 392               # conv i-block (psum width)
NIB = HALF // IB       # 4
PHW = 2 * IB           # 784 p-tile width (stride %16 == 0 for DoubleRow)
JSLICES = [256, 480, 480, 480, 480, 480, 480]   # G-phase j-slices (sum 3136)
NJS = len(JSLICES)
NIT = 13               # i-tiles: 12 full + 1 ragged(32)
LAST_W = HALF - 12 * 128   # 32
S_P, S_W, S_PHI, S_C = 16.0, 1024.0, 16.0, 512.0
SYS = S_PHI * S_C      # psum = SYS * (2 phi.c - c)
N_WARM = 24

_cache = {}


def _build_program():
    import concourse.tile as tile
    from concourse import bacc, mybir

    F32 = mybir.dt.float32
    F32R = mybir.dt.float32r
    FP8 = mybir.dt.float8e4
    AF = mybir.ActivationFunctionType
    ALU = mybir.AluOpType
    AX = mybir.AxisListType
    DR = mybir.MatmulPerfMode.DoubleRow

    nc = bacc.Bacc("TRN2", target_bir_lowering=False, debug=False)

    pT_d = nc.dram_tensor("pT", [DIM, HALF], FP8, kind="ExternalInput")
    wt_d = nc.dram_tensor("wt", [DIM, DIM], FP8, kind="ExternalInput")   # W^T
    cb_d = nc.dram_tensor("cb", [KCB * 128, P], FP8, kind="ExternalInput")
    bias16_d = nc.dram_tensor("bias16", [DIM], F32, kind="ExternalInput")
    onec_d = nc.dram_tensor("onec", [128, 1], F32R, kind="ExternalInput")
    oner_d = nc.dram_tensor("oner", [1, 128], F32R, kind="ExternalInput")
    score_d = nc.dram_tensor("score", [128, NIT], F32, kind="ExternalOutput")

    with tile.TileContext(nc) as tc:
        with (
            tc.tile_pool(name="persist", bufs=1) as persist,
            tc.tile_pool(name="cbp0", bufs=1) as cbp0,
        ):
            phi = persist.tile([128, KCB, HALF], FP8)
            bias16_col = persist.tile([128, KC], F32)
            onec = persist.tile([128, 1], F32R)
            oner = persist.tile([1, 128], F32R)
            f_row = persist.tile([1, HALF], F32)
            f_col = persist.tile([128, NIT], F32)
            runAll = persist.tile([128, NIT, NJS, 8], F32)
            runF = persist.tile([128, NIT, 8], F32)
            score_col = persist.tile([128, NIT], F32)

            # constant correction pair: phi planes 14/15 = 16.0 (gpsimd; idle)
            nc.gpsimd.memset(phi[:, KC:KCB, :], 16.0)

            # ------------- conv phase: phi = W @ p + b, f = ||phi||^2 -------
            with (
                tc.tile_pool(name="pp", bufs=2) as pp,
                tc.tile_pool(name="wtp", bufs=3) as wtp,
                tc.tile_pool(name="sqp", bufs=4) as sqp,
                tc.tile_pool(name="cps", bufs=4, space="PSUM") as cps,
                tc.tile_pool(name="fps", bufs=1, space="PSUM") as fps,
            ):
                f_ps = [fps.tile([1, IB], F32, name=f"fp{ib}", tag=f"f{ib}")
                        for ib in range(NIB)]

                def load_wt(dcg):
                    t = wtp.tile([128, KC, 128], FP8, name="wt_t")
                    nc.sync.dma_start(
                        t[:],
                        wt_d[:, dcg * 128:(dcg + 1) * 128].rearrange(
                            "(cc p) d -> p cc d", p=128),
                    )
                    return t

                def load_p(h):
                    t = pp.tile([128, KC, PHW], FP8, name=f"pq{h}", tag="pq")
                    nc.sync.dma_start(
                        t[:],
                        pT_d[:, h * PHW:(h + 1) * PHW].rearrange(
                            "(cc p) i -> p cc i", p=128),
                    )
                    return t

                # startup-critical loads first: wt chunk 0, then p tile 0
                dcg_seq = list(range(KC)) + list(reversed(range(KC)))  # snake
                wt_tiles = {0: load_wt(dcg_seq[0])}
                wt_issued = 1

                def wt_prefetch(upto):
                    nonlocal wt_issued
                    while wt_issued < min(upto, 2 * KC):
                        if dcg_seq[wt_issued] == dcg_seq[wt_issued - 1]:
                            # snake turn: same chunk again, reuse the tile
                            wt_tiles[wt_issued] = wt_tiles[wt_issued - 1]
                        else:
                            wt_tiles[wt_issued] = load_wt(dcg_seq[wt_issued])
                        wt_issued += 1

                # PE warmup: dummy matmuls keep HAM's activity monitor hot
                # while the first real DMAs land, so conv starts at 2.4 GHz.
                warm = pp.tile([128, 512], F32R, name="warm", tag="warm", bufs=1)
                nc.vector.memset(warm[:].bitcast(F32), 1.0)
                wps = cps.tile([128, 512], F32, name="wps", tag="acc")
                for _ in range(N_WARM):
                    nc.tensor.matmul(wps[:], warm[:, 0:128], warm[:],
                                     start=True, stop=True)

                cb0_t = None
                small_dmas_done = False
                pending_f = []
                for sub in range(2):
                    p_t = load_p(sub)
                    if not small_dmas_done:
                        small_dmas_done = True
                        nc.sync.dma_start(
                            bias16_col[:],
                            bias16_d.rearrange("(g p) -> p g", p=128))
                        nc.sync.dma_start(onec[:], onec_d[:])
                        nc.sync.dma_start(oner[:], oner_d[:])
                    for dcg_i in range(KC):
                        pos = sub * KC + dcg_i
                        dcg = dcg_seq[pos]
                        wt_t = wt_tiles.pop(pos)
                        wt_prefetch(pos + 3)
                        for k, ib in enumerate((2 * sub, 2 * sub + 1)):
                            if k == 1 and pending_f:
                                # deferred f matmuls: deps long satisfied
                                for args, kw in pending_f:
                                    nc.tensor.matmul(*args, **kw)
                                pending_f = []
                            ioff = (ib % 2) * IB
                            acc = cps.tile([128, IB], F32)
                            for c in range(KPAIR):
                                nc.tensor.matmul(
                                    acc[:],
                                    wt_t[:, 2 * c:2 * c + 2, :],
                                    p_t[:, 2 * c:2 * c + 2, ioff:ioff + IB],
                                    start=(c == 0),
                                    stop=(c == KPAIR - 1),
                                    perf_mode=DR,
                                )
                            isl = slice(ib * IB, (ib + 1) * IB)
                            # phi8 = (psum/1024) + 16 b   (= 16*phi, fp8)
                            nc.scalar.activation(
                                phi[:, dcg, isl], acc[:], AF.Identity,
                                bias=bias16_col[:, dcg:dcg + 1],
                                scale=1.0 / 1024.0,
                            )
                            # sq = phi8^2 = 256*phi^2 (DVE; consistent f)
                            sq = sqp.tile([128, IB], F32R)
                            nc.vector.tensor_tensor(
                                sq[:], phi[:, dcg, isl],
                                phi[:, dcg, isl], ALU.mult,
                            )
                            pending_f.append((
                                (f_ps[ib][:], onec[:], sq[:]),
                                dict(start=(dcg_i == 0), stop=(dcg_i == KC - 1)),
                            ))
                    if sub == 0:
                        # prefetch first G slice mid-conv
                        j0 = JSLICES[0]
                        cb0_t = cbp0.tile([128, KCB, j0], FP8)
                        nc.sync.dma_start(
                            cb0_t[:],
                            cb_d[:, 0:j0].rearrange("(cc p) j -> p cc j",
                                                    p=128),
                        )
                for args, kw in pending_f:
                    nc.tensor.matmul(*args, **kw)
                pending_f = []
                for ib in range(NIB):
                    nc.vector.tensor_copy(
                        f_row[:, ib * IB:(ib + 1) * IB], f_ps[ib][:]
                    )

            # ------------- f relayout: [1, 1568] -> [128, 13] ---------------
            with tc.tile_pool(name="ftp", bufs=2, space="PSUM") as ftp:
                ft = ftp.tile([128, NIT], F32)
                for it in range(NIT):
                    w = 128 if it < 12 else LAST_W
                    nc.tensor.transpose(
                        ft[0:w, it:it + 1],
                        f_row[:, it * 128:it * 128 + w],
                        oner[0:1, 0:1].bitcast(F32),
                    )
                # f_col = f (sq was 256*phi^2)
                nc.scalar.activation(f_col[:], ft[:], AF.Copy,
                                     scale=1.0 / 256.0)

            # ------------- G phase: psum = 8192*(2 phi.c - c), top-8 --------
            with (
                tc.tile_pool(name="cbp", bufs=2) as cbp,
                tc.tile_pool(name="yps", bufs=8, space="PSUM") as yps,
            ):
                joff = [0]
                for js in range(1, NJS):
                    joff.append(joff[-1] + JSLICES[js - 1])

                for js in range(NJS):
                    w_js = JSLICES[js]
                    jsl = slice(joff[js], joff[js] + w_js)
                    if js == 0:
                        cb_t = cb0_t
                    else:
                        cb_t = cbp.tile([128, KCB, w_js], FP8, name="cb_t",
                                        tag="cb")
                        nc.sync.dma_start(
                            cb_t[:],
                            cb_d[:, jsl].rearrange("(cc p) j -> p cc j",
                                                   p=128),
                        )
                    for it in range(NIT):
                        w = 128 if it < 12 else LAST_W
                        i0 = it * 128
                        y = yps.tile([128, 512], F32, name="y", tag="y")
                        for c in range(GPAIR):
                            nc.tensor.matmul(
                                y[0:w, 0:w_js],
                                phi[:, 2 * c:2 * c + 2, i0:i0 + w],
                                cb_t[:, 2 * c:2 * c + 2, :],
                                start=(c == 0),
                                stop=(c == GPAIR - 1),
                                perf_mode=DR,
                            )
                        nc.vector.max(runAll[0:w, it, js, :], y[0:w, 0:w_js])

                # merge the per-window top-8s, then sqrt + softmin weight
                with tc.tile_pool(name="tails", bufs=4) as tails:
                    for it in range(NIT):
                        w = 128 if it < 12 else LAST_W
                        nc.vector.max(runF[0:w, it, :], runAll[0:w, it, :, :])
                        d3 = tails.tile([128, 3], F32, tag="d3")
                        nc.scalar.activation(
                            d3[0:w, :], runF[0:w, it, 0:3], AF.Sqrt,
                            bias=f_col[0:w, it:it + 1], scale=-1.0 / SYS,
                        )
                        dd = tails.tile([128, 3], F32, tag="dd")
                        nc.vector.tensor_scalar(
                            dd[0:w, :], d3[0:w, :], d3[0:w, 0:1], None,
                            ALU.subtract,
                        )
                        ee = tails.tile([128, 3], F32, tag="ee")
                        nc.scalar.activation(ee[0:w, :], dd[0:w, :], AF.Exp,
                                             scale=-1.0)
                        ss = tails.tile([128, 1], F32, tag="ss")
                        nc.vector.tensor_reduce(ss[0:w, :], ee[0:w, :], AX.X,
                                                ALU.add)
                        rr = tails.tile([128, 1], F32, tag="rr")
                        nc.vector.reciprocal(rr[0:w, :], ss[0:w, :])
                        nc.vector.tensor_scalar(
                            score_col[0:w, it:it + 1], d3[0:w, 0:1],
                            rr[0:w, 0:1], None, ALU.mult,
                        )
            nc.sync.dma_start(score_d[:], score_col[:])

    nc.compile()
    return nc


def _get_program():
    if "nc" not in _cache:
        _cache["nc"] = _build_program()
    return _cache["nc"]


def kernel(p, W, b, C):
    import ml_dtypes
    from concourse.bass_utils import run_bass_kernel_spmd

    E4 = ml_dtypes.float8_e4m3

    nc = _get_program()

    p = np.asarray(p, dtype=np.float32)
    W = np.asarray(W, dtype=np.float32)
    b = np.ascontiguousarray(np.asarray(b, dtype=np.float32))
    C = np.asarray(C, dtype=np.float32)

    wt8 = np.ascontiguousarray(W.T * np.float32(S_W)).astype(E4)      # [c, d]
    cn = np.sum(C.astype(np.float64) * C, axis=0)
    corr8 = np.asarray(-2.0 * cn, dtype=np.float32).astype(E4)        # [P]
    cb = np.empty((KCB * 128, P), dtype=E4)
    cb[:DIM] = (C * np.float32(2.0 * S_C)).astype(E4)
    cb[DIM:DIM + 128] = corr8[None, :]
    cb[DIM + 128:] = corr8[None, :]
    cb = np.ascontiguousarray(cb)
    bias16 = np.ascontiguousarray(b * np.float32(S_PHI))
    onec = np.ones((128, 1), dtype=np.float32)
    oner = np.ones((1, 128), dtype=np.float32)

    p8 = (p.reshape(B, DIM, HW) * np.float32(S_P)).astype(E4)
    in_maps = []
    for core in range(NCORES):
        bidx, half = divmod(core, 2)
        pT = np.ascontiguousarray(p8[bidx, :, half * HALF:(half + 1) * HALF])
        in_maps.append({
            "pT": pT, "wt": wt8, "cb": cb, "bias16": bias16,
            "onec": onec, "oner": oner,
        })

    _cache["last_in_maps"] = in_maps
    res = run_bass_kernel_spmd(nc, in_maps, list(range(NCORES)))
    _cache["last_result"] = res

    return assemble_output(per_core=[res.results[c]["score"] for c in range(NCORES)])


def assemble_output(per_core=None, res_concat=None):
    if per_core is None:
        sc_all = res_concat["score"]                              # [8*128, 13]
        per_core = [sc_all[c * 128:(c + 1) * 128] for c in range(NCORES)]
    out = np.empty((B, 1, H, W_), dtype=np.float32)
    for core in range(NCORES):
        bidx, half = divmod(core, 2)
        sc = per_core[core]                                       # [128, 13]
        flat = np.empty(HALF, dtype=np.float32)
        flat[:12 * 128] = sc[:, :12].T.reshape(-1)
        flat[12 * 128:] = sc[:LAST_W, 12]
        out.reshape(B, 1, HW)[bidx, 0, half * HALF:(half + 1) * HALF] = flat
    return out


# revision 10
# speedup vs baseline: 1.9729x; 1.0971x over previous
"""Trainium2 Bass kernel for nn_DSVDD (retrieval_knn), fp8 DoubleRow version.

Math (per batch b):
  phi = W @ p_b + bias            [DIM, HW]    (1x1 conv)
  sqdist[i,j] = ||phi_i||^2 + ||C_j||^2 - 2 phi_i . C_j
  top-3 smallest distances d0<=d1<=d2  ->  w0 = 1/(1+exp(d0-d1)+exp(d0-d2))
  score[i] = w0 * d0

Device strategy (8 cores, data-parallel over (batch, HW-half)):
  Both GEMMs run as fp8e4 DoubleRow matmuls (2 MAC/PE/cycle): host quantizes
  p*16, W*1024, 2C*512 to e4m3 (all well under the TRN 240 cap).  The
  -8192*||c_j||^2 correction is pre-written (exact, f32) into each distance
  psum tile by an ACT copy before the 7 DoubleRow pair-matmuls accumulate
  on top (start=False), so psum = 8192*Y with Y = 2 phi.c - c and no PE or
  DVE cycles go to the correction.  top-3 smallest sqdist == top-3 largest
  Y (f_i common per row).  DVE max8 collects the top-8 of each 256..480-wide
  j-window into runAll; one final max8 per i-tile merges the windows.
  f_i = ||phi||^2 comes from DVE squares of the quantized phi (consistent-f)
  reduced by ones-matmuls, deferred one conv step so they never stall the
  PE.  Tail (sqrt, softmin) on ACT/DVE, batched per function so the ACT
  table is not reloaded per i-tile.
"""
import sys

sys.path.insert(0, "/opt/trn_rl_repo")

import numpy as np

B, DIM, H, W_ = 4, 1792, 56, 56
HW = H * W_            # 3136
P = 3136               # prototypes
NCORES = 8
HALF = HW // 2         # 1568 positions per core
KC = DIM // 128        # 14 contraction chunks
KPAIR = KC // 2        # 7 DoubleRow pairs in both GEMMs
IB = 392               # conv i-block (psum width)
NIB = HALF // IB       # 4
PHW = 2 * IB           # 784 p-tile width (stride %16 == 0 for DoubleRow)
JSLICES = [256, 480, 480, 480, 480, 480, 480]   # G-phase j-slices (sum 3136)
NJS = len(JSLICES)
NIT = 13               # i-tiles: 12 full + 1 ragged(32)
LAST_W = HALF - 12 * 128   # 32
S_P, S_W, S_PHI, S_C = 16.0, 1024.0, 16.0, 512.0
SYS = S_PHI * S_C      # psum = SYS * (2 phi.c - c)
N_WARM = 24

_cache = {}


def _build_program():
    import concourse.tile as tile
    from concourse import bacc, mybir

    F32 = mybir.dt.float32
    F32R = mybir.dt.float32r
    FP8 = mybir.dt.float8e4
    AF = mybir.ActivationFunctionType
    ALU = mybir.AluOpType
    AX = mybir.AxisListType
    DR = mybir.MatmulPerfMode.DoubleRow

    nc = bacc.Bacc("TRN2", target_bir_lowering=False, debug=False)

    pT_d = nc.dram_tensor("pT", [DIM, HALF], FP8, kind="ExternalInput")
    wt_d = nc.dram_tensor("wt", [DIM, DIM], FP8, kind="ExternalInput")   # W^T
    cb_d = nc.dram_tensor("cb", [KC * 128, P], FP8, kind="ExternalInput")
    cbc_d = nc.dram_tensor("cbc", [128, P], F32, kind="ExternalInput")
    bias16_d = nc.dram_tensor("bias16", [DIM], F32, kind="ExternalInput")
    onec_d = nc.dram_tensor("onec", [128, 1], F32R, kind="ExternalInput")
    oner_d = nc.dram_tensor("oner", [1, 128], F32R, kind="ExternalInput")
    score_d = nc.dram_tensor("score", [128, NIT], F32, kind="ExternalOutput")

    with tile.TileContext(nc) as tc:
        with (
            tc.tile_pool(name="persist", bufs=1) as persist,
            tc.tile_pool(name="cbp0", bufs=1) as cbp0,
        ):
            phi = persist.tile([128, KC, HALF], FP8)
            bias16_col = persist.tile([128, KC], F32)
            onec = persist.tile([128, 1], F32R)
            oner = persist.tile([1, 128], F32R)
            f_row = persist.tile([1, HALF], F32)
            f_col = persist.tile([128, NIT], F32)
            runAll = persist.tile([128, NIT, NJS, 8], F32)
            runF = persist.tile([128, NIT, 8], F32)
            score_col = persist.tile([128, NIT], F32)

            # ------------- conv phase: phi = W @ p + b, f = ||phi||^2 -------
            with (
                tc.tile_pool(name="pp", bufs=2) as pp,
                tc.tile_pool(name="wtp", bufs=3) as wtp,
                tc.tile_pool(name="sqp", bufs=4) as sqp,
                tc.tile_pool(name="cps", bufs=4, space="PSUM") as cps,
                tc.tile_pool(name="fps", bufs=1, space="PSUM") as fps,
            ):
                f_ps = [fps.tile([1, IB], F32, name=f"fp{ib}", tag=f"f{ib}")
                        for ib in range(NIB)]

                def load_wt(dcg):
                    t = wtp.tile([128, KC, 128], FP8, name="wt_t")
                    nc.sync.dma_start(
                        t[:],
                        wt_d[:, dcg * 128:(dcg + 1) * 128].rearrange(
                            "(cc p) d -> p cc d", p=128),
                    )
                    return t

                def load_p(h):
                    t = pp.tile([128, KC, PHW], FP8, name=f"pq{h}", tag="pq")
                    nc.sync.dma_start(
                        t[:],
                        pT_d[:, h * PHW:(h + 1) * PHW].rearrange(
                            "(cc p) i -> p cc i", p=128),
                    )
                    return t

                # startup-critical loads first: wt chunk 0, then p tile 0
                dcg_seq = list(range(KC)) + list(reversed(range(KC)))  # snake
                wt_tiles = {0: load_wt(dcg_seq[0])}
                wt_issued = 1

                def wt_prefetch(upto):
                    nonlocal wt_issued
                    while wt_issued < min(upto, 2 * KC):
                        if dcg_seq[wt_issued] == dcg_seq[wt_issued - 1]:
                            # snake turn: same chunk again, reuse the tile
                            wt_tiles[wt_issued] = wt_tiles[wt_issued - 1]
                        else:
                            wt_tiles[wt_issued] = load_wt(dcg_seq[wt_issued])
                        wt_issued += 1

                # PE warmup: dummy matmuls keep HAM's activity monitor hot
                # while the first real DMAs land, so conv starts at 2.4 GHz.
                warm = pp.tile([128, 512], F32R, name="warm", tag="warm", bufs=1)
                nc.vector.memset(warm[:].bitcast(F32), 1.0)
                wps = cps.tile([128, 512], F32, name="wps", tag="acc")
                for _ in range(N_WARM):
                    nc.tensor.matmul(wps[:], warm[:, 0:128], warm[:],
                                     start=True, stop=True)

                cb0_t = None
                small_dmas_done = False
                pending_f = []
                for sub in range(2):
                    p_t = load_p(sub)
                    if not small_dmas_done:
                        small_dmas_done = True
                        nc.sync.dma_start(
                            bias16_col[:],
                            bias16_d.rearrange("(g p) -> p g", p=128))
                        nc.sync.dma_start(onec[:], onec_d[:])
                        nc.sync.dma_start(oner[:], oner_d[:])
                    for dcg_i in range(KC):
                        pos = sub * KC + dcg_i
                        dcg = dcg_seq[pos]
                        wt_t = wt_tiles.pop(pos)
                        wt_prefetch(pos + 3)
                        for k, ib in enumerate((2 * sub, 2 * sub + 1)):
                            if k == 1 and pending_f:
                                # deferred f matmuls: deps long satisfied
                                for args, kw in pending_f:
                                    nc.tensor.matmul(*args, **kw)
                                pending_f = []
                            ioff = (ib % 2) * IB
                            acc = cps.tile([128, IB], F32)
                            for c in range(KPAIR):
                                nc.tensor.matmul(
                                    acc[:],
                                    wt_t[:, 2 * c:2 * c + 2, :],
                                    p_t[:, 2 * c:2 * c + 2, ioff:ioff + IB],
                                    start=(c == 0),
                                    stop=(c == KPAIR - 1),
                                    perf_mode=DR,
                                )
                            isl = slice(ib * IB, (ib + 1) * IB)
                            # phi8 = (psum/1024) + 16 b   (= 16*phi, fp8)
                            nc.scalar.activation(
                                phi[:, dcg, isl], acc[:], AF.Identity,
                                bias=bias16_col[:, dcg:dcg + 1],
                                scale=1.0 / 1024.0,
                            )
                            # sq = phi8^2 = 256*phi^2 (DVE; consistent f)
                            sq = sqp.tile([128, IB], F32R)
                            nc.vector.tensor_tensor(
                                sq[:], phi[:, dcg, isl],
                                phi[:, dcg, isl], ALU.mult,
                            )
                            pending_f.append((
                                (f_ps[ib][:], onec[:], sq[:]),
                                dict(start=(dcg_i == 0), stop=(dcg_i == KC - 1)),
                            ))
                    if sub == 0:
                        # prefetch first G slice (+ its correction) mid-conv
                        j0 = JSLICES[0]
                        cb0_t = cbp0.tile([128, KC, j0], FP8)
                        nc.sync.dma_start(
                            cb0_t[:],
                            cb_d[:, 0:j0].rearrange("(cc p) j -> p cc j",
                                                    p=128),
                        )
                        cbc0_t = cbp0.tile([128, JSLICES[0]], F32, name="cbc0")
                        nc.sync.dma_start(cbc0_t[:], cbc_d[:, 0:j0])
                for args, kw in pending_f:
                    nc.tensor.matmul(*args, **kw)
                pending_f = []
                for ib in range(NIB):
                    nc.vector.tensor_copy(
                        f_row[:, ib * IB:(ib + 1) * IB], f_ps[ib][:]
                    )

            # ------------- f relayout: [1, 1568] -> [128, 13] ---------------
            with tc.tile_pool(name="ftp", bufs=2, space="PSUM") as ftp:
                ft = ftp.tile([128, NIT], F32)
                for it in range(NIT):
                    w = 128 if it < 12 else LAST_W
                    nc.tensor.transpose(
                        ft[0:w, it:it + 1],
                        f_row[:, it * 128:it * 128 + w],
                        oner[0:1, 0:1].bitcast(F32),
                    )
                # f_col = f (sq was 256*phi^2)
                nc.scalar.activation(f_col[:], ft[:], AF.Copy,
                                     scale=1.0 / 256.0)

            # ------------- G phase: psum = 8192*(2 phi.c - c), top-8 --------
            with (
                tc.tile_pool(name="cbp", bufs=2) as cbp,
                tc.tile_pool(name="cbcp", bufs=2) as cbcp,
                tc.tile_pool(name="yps", bufs=8, space="PSUM") as yps,
            ):
                joff = [0]
                for js in range(1, NJS):
                    joff.append(joff[-1] + JSLICES[js - 1])

                for js in range(NJS):
                    w_js = JSLICES[js]
                    jsl = slice(joff[js], joff[js] + w_js)
                    if js == 0:
                        cb_t = cb0_t
                        cbc_t = cbc0_t
                    else:
                        cb_t = cbp.tile([128, KC, w_js], FP8, name="cb_t",
                                        tag="cb")
                        nc.sync.dma_start(
                            cb_t[:],
                            cb_d[:, jsl].rearrange("(cc p) j -> p cc j",
                                                   p=128),
                        )
                        cbc_t = cbcp.tile([128, 512], F32, name="cbc_t",
                                          tag="cbc")
                        nc.sync.dma_start(cbc_t[:, 0:w_js], cbc_d[:, jsl])
                    for it in range(NIT):
                        w = 128 if it < 12 else LAST_W
                        i0 = it * 128
                        y = yps.tile([128, 512], F32, name="y", tag="y")
                        # exact -8192*||c_j||^2 pre-written into the psum
                        nc.scalar.activation(y[0:w, 0:w_js],
                                             cbc_t[0:w, 0:w_js], AF.Copy)
                        for c in range(KPAIR):
                            nc.tensor.matmul(
                                y[0:w, 0:w_js],
                                phi[:, 2 * c:2 * c + 2, i0:i0 + w],
                                cb_t[:, 2 * c:2 * c + 2, :],
                                start=False,
                                stop=(c == KPAIR - 1),
                                perf_mode=DR,
                            )
                        nc.vector.max(runAll[0:w, it, js, :], y[0:w, 0:w_js])

                # merge the per-window top-8s, then sqrt + softmin weight;
                # batched per function so the ACT table loads only twice.
                with tc.tile_pool(name="tails", bufs=1) as tails:
                    d3s = tails.tile([128, NIT, 3], F32)
                    dds = tails.tile([128, NIT, 3], F32)
                    ees = tails.tile([128, NIT, 3], F32)
                    sss = tails.tile([128, NIT], F32)
                    rrs = tails.tile([128, NIT], F32)
                    for it in range(NIT):
                        w = 128 if it < 12 else LAST_W
                        nc.vector.max(runF[0:w, it, :], runAll[0:w, it, :, :])
                    for it in range(NIT):
                        w = 128 if it < 12 else LAST_W
                        nc.scalar.activation(
                            d3s[0:w, it, :], runF[0:w, it, 0:3], AF.Sqrt,
                            bias=f_col[0:w, it:it + 1], scale=-1.0 / SYS,
                        )
                    for it in range(NIT):
                        w = 128 if it < 12 else LAST_W
                        nc.vector.tensor_scalar(
                            dds[0:w, it, :], d3s[0:w, it, :],
                            d3s[0:w, it, 0:1], None, ALU.subtract,
                        )
                    nc.scalar.activation(ees[:], dds[:], AF.Exp, scale=-1.0)
                    nc.vector.tensor_reduce(sss[:], ees[:], AX.X, ALU.add)
                    nc.vector.reciprocal(rrs[:], sss[:])
                    nc.vector.tensor_tensor(score_col[:], d3s[:, :, 0],
                                            rrs[:], ALU.mult)
            nc.sync.dma_start(score_d[:], score_col[:])

    nc.compile()
    return nc


def _get_program():
    if "nc" not in _cache:
        _cache["nc"] = _build_program()
    return _cache["nc"]


def kernel(p, W, b, C):
    import ml_dtypes
    from concourse.bass_utils import run_bass_kernel_spmd

    E4 = ml_dtypes.float8_e4m3

    nc = _get_program()

    p = np.asarray(p, dtype=np.float32)
    W = np.asarray(W, dtype=np.float32)
    b = np.ascontiguousarray(np.asarray(b, dtype=np.float32))
    C = np.asarray(C, dtype=np.float32)

    wt8 = np.ascontiguousarray(W.T * np.float32(S_W)).astype(E4)      # [c, d]
    cn = np.sum(C.astype(np.float64) * C, axis=0)
    cb = np.ascontiguousarray((C * np.float32(2.0 * S_C)).astype(E4))
    cbc = np.ascontiguousarray(np.broadcast_to(
        (-SYS * cn).astype(np.float32)[None, :], (128, P)))
    bias16 = np.ascontiguousarray(b * np.float32(S_PHI))
    onec = np.ones((128, 1), dtype=np.float32)
    oner = np.ones((1, 128), dtype=np.float32)

    p8 = (p.reshape(B, DIM, HW) * np.float32(S_P)).astype(E4)
    in_maps = []
    for core in range(NCORES):
        bidx, half = divmod(core, 2)
        pT = np.ascontiguousarray(p8[bidx, :, half * HALF:(half + 1) * HALF])
        in_maps.append({
            "pT": pT, "wt": wt8, "cb": cb, "cbc": cbc, "bias16": bias16,
            "onec": onec, "oner": oner,
        })

    _cache["last_in_maps"] = in_maps
    res = run_bass_kernel_spmd(nc, in_maps, list(range(NCORES)))
    _cache["last_result"] = res

    return assemble_output(per_core=[res.results[c]["score"] for c in range(NCORES)])


def assemble_output(per_core=None, res_concat=None):
    if per_core is None:
        sc_all = res_concat["score"]                              # [8*128, 13]
        per_core = [sc_all[c * 128:(c + 1) * 128] for c in range(NCORES)]
    out = np.empty((B, 1, H, W_), dtype=np.float32)
    for core in range(NCORES):
        bidx, half = divmod(core, 2)
        sc = per_core[core]                                       # [128, 13]
        flat = np.empty(HALF, dtype=np.float32)
        flat[:12 * 128] = sc[:, :12].T.reshape(-1)
        flat[12 * 128:] = sc[:LAST_W, 12]
        out.reshape(B, 1, HW)[bidx, 0, half * HALF:(half + 1) * HALF] = flat
    return out
